# revision 32
# baseline (speedup 1.0000x reference)
"""Batch-parallel dot-product attention for TRN2 (8 NeuronCores).

reference: context[b] = softmax(Q[b] @ K[b].T / sqrt(64)) @ V[b]
with Q,K,V: [32, 2048, 64] fp32.

Sharding: pure data parallel - 4 batches per core, no collectives.

Per-core kernel, per (batch, 1024-query half), 16 key-tile steps:
  sc[k, q]  = (K_t @ Q^T)         one 1024-wide matmul pair per k-tile (PE)
  pt        = exp(sc/8) fp16      ACT for 13/16 steps; 3/16 steps use a
                                  7-op DVE exp2 bit-trick chain so the two
                                  engines' exp throughput adds up
  cx[d, q] += Vaug_t^T @ pt       PSUM accumulation, Vaug = [V | 1]
  (rows 64.. of cx = softmax denominator via the ones columns)

Engine layout (the whole point of this schedule):
  PE   : QK + AV matmuls only, zero-gap stream (keeps 2.4GHz p-state)
  ACT  : exp on 13/16 steps
  DVE  : exp chain on 3/16 steps + tiny reciprocal in the drain
  Pool : all drain compute (cx->SBUF fp16 copy, per-chunk normalize) + DMA
  SP   : input DMAs + xbar transposes

PSUM: sc ping-pong (2x2 banks) + cx ping-pong (2x2 banks) = 8 banks, so
AV of the next (b,h) overlaps the previous drain with no PE stall.
AV matmuls are deferred (ACT steps: 3, DVE steps: 7 global steps) so PE
never waits on the exp producing their rhs.
"""

import numpy as np

import concourse.bass as bass
import concourse.bacc as bacc
import concourse.tile as tile
from concourse import mybir
from concourse.bass_utils import run_bass_kernel_spmd

NCORES = 8
BPC = 4  # batches per core
S = 2048
D = 64
DA = 96  # V augmented to 96 cols (64 V + 32 ones) for 32-aligned xbar transpose
NKT = S // 128  # 16 key tiles of 128
NH = 2  # query halves
HQ = S // NH  # 1024 queries per half

# steps (k-tile indices) per (b,h) whose exp runs on DVE instead of ACT.
# 2/16 steps on DVE keeps ACT (14 x 996ns = 13.9us) just above the PE
# window (13.65us) while leaving DVE mostly idle so its 4.7us chains
# never congest; more DVE steps trade ACT slack for queue fragility.
DVE_KS = (4, 10)
AV_DELAY_ACT = 3  # AV of an ACT step is emitted this many global steps later
AV_DELAY_LATE = 2  # steps 14/15: release cx sooner so the drain can start
AV_DELAY_DVE = 7  # DVE chain is longer; its AVs defer further

FP16 = mybir.dt.float16
F32 = mybir.dt.float32
I16 = mybir.dt.int16

# fp16 exp2 bit-trick constants (DVE offload path): exp(s/8) = 2^y,
# y = s*log2(e)/8; t = fp16(y + 1536) rounds y to int n = t - 1536;
# s16 = t*1024 + SBIAS = fp16 bits of 2^n; g = n - y in [-0.5, 0.5];
# 2^-g ~ C0 - C1*g + C2*g^2 (minimax, rel err 2.2e-3).
LOG2E_8 = 1.4426950408889634 / 8.0
MAGIC = 1536.0
SBIAS = 15360.0 - MAGIC * 1024.0
C0, C1, C2 = 1.00053068, 0.70552215, 0.23946112

_cache = {}


def _build(reps=1):
    if reps in _cache:
        return _cache[reps]

    nc = bacc.Bacc(
        "TRN2",
        target_bir_lowering=False,
        debug=False,
        num_devices=1,
        enable_partition_id=False,
    )

    qt_d = nc.dram_tensor("qt", [BPC, D, S], FP16, kind="ExternalInput").ap()
    kt_d = nc.dram_tensor("kt", [BPC, D, S], FP16, kind="ExternalInput").ap()
    # host pre-tiles V-augmented to [BPC, 128, NKT, DA] so the DMA is contiguous
    va_d = nc.dram_tensor("va", [BPC, 128, NKT, DA], FP16, kind="ExternalInput").ap()
    # device writes [BPC, NH, 128, 8*D] contiguously; host re-tiles to [B, S, D]
    out_d = nc.dram_tensor("out", [BPC, NH, 128, 8 * D], FP16, kind="ExternalOutput").ap()

    with tile.TileContext(nc) as tc:
        with (
            tc.tile_pool(name="io", bufs=1) as io,
            tc.tile_pool(name="pt", bufs=13) as ptp,
            tc.tile_pool(name="csb", bufs=2) as csbp,
            tc.tile_pool(name="outsb", bufs=2) as outp,
            tc.tile_pool(name="dvet", bufs=3) as dvet,
            tc.tile_pool(name="scps", bufs=3, space="PSUM") as scps,
            tc.tile_pool(name="cxps", bufs=1, space="PSUM") as cxps,
        ):

            def dve_exp_tail(y, pt):
                """pt = 2^y given y (fp16 SBUF), all on DVE (tensor_scalar
                runs 4x, tensor_tensor 2x; scalar_tensor_tensor would be 1x
                so it's avoided; cross-engine hops put the drain's latency
                chain in front of pt and stall PE, so none of those).

                t rounds y to n = t - MAGIC; s16 = fp16 bits of 2^n;
                f = y - n; pt = (C2 f^2 + C1 f + C0) * 2^n.
                """
                t = dvet.tile([128, HQ], FP16, name="t16")
                nc.vector.tensor_scalar_add(t, y, MAGIC)
                n = dvet.tile([128, HQ], FP16, name="n16")
                nc.vector.tensor_scalar_sub(n, t, MAGIC)
                s16 = dvet.tile([128, HQ], I16, name="s16")
                nc.vector.tensor_scalar(
                    s16, t, 1024.0, SBIAS,
                    op0=mybir.AluOpType.mult, op1=mybir.AluOpType.add,
                )
                f = dvet.tile([128, HQ], FP16, name="f16")
                nc.vector.tensor_tensor(f, y, n, op=mybir.AluOpType.subtract)
                a = dvet.tile([128, HQ], FP16, name="a16")
                nc.vector.tensor_scalar(
                    a, f, C2, C1,
                    op0=mybir.AluOpType.mult, op1=mybir.AluOpType.add,
                )
                nc.vector.tensor_tensor(a, a, f, op=mybir.AluOpType.mult)
                nc.vector.tensor_scalar_add(a, a, C0)
                nc.vector.tensor_tensor(
                    pt, a, s16.bitcast(FP16), op=mybir.AluOpType.mult
                )

            def drain(cx, b, h, last=False):
                # split into small closures; one is emitted per k-step so
                # the work interleaves with the next half's steps. Every
                # piece is a strictly per-chunk chain (copy-half -> its
                # transposes -> per-chunk recip+norm) with no global join,
                # so no engine queue ever head-of-line blocks on a late
                # cross-engine fan-in.
                state = {}

                def start():
                    # cx -> SBUF fp16 in two halves on DVE (GPSIMD compute
                    # cannot touch PSUM on HW; ACT is the busy engine). cx
                    # is single-buffered, so the next half's first AV waits
                    # on this copy - the halves let chunk 0-3 transposes
                    # start after the first one lands.
                    state["csb"] = csbp.tile([DA, HQ], FP16, name="csb")
                    state["out_sb"] = outp.tile([128, 8 * D], FP16, name="out_sb")
                    state["ct"] = csbp.tile([128, 8 * DA], FP16, name="ctT")
                    state["r8"] = outp.tile([128, 8], F32, name="r8")
                    nc.vector.tensor_copy(state["csb"][:, 0:512], cx[:, 0:512])
                    nc.vector.tensor_copy(state["csb"][:, 512:HQ], cx[:, 512:HQ])
                    # reciprocal of the denominator row BEFORE the transpose
                    # (rows D..DA of csb are identical denom copies): one
                    # [1,1024] op; the transposes then carry 1/denom so the
                    # normalize is Pool-only with per-chunk deps. Doing this
                    # per-chunk after the transpose paced the DVE queue at
                    # transpose speed and convoyed the exp chains.
                    with nc.allow_low_precision(
                        reason="fp16 1/denom; denom O(3e3) well inside fp16, "
                        "rel err ~5e-4 vs 2e-2 gate"
                    ):
                        # in-place on row D (partition base must be 32-
                        # aligned, so row D+1 is not addressable); two
                        # halves so chunk 0-3 transposes start sooner
                        nc.vector.reciprocal(
                            state["csb"][D : D + 1, 0:512],
                            state["csb"][D : D + 1, 0:512],
                        )
                        nc.vector.reciprocal(
                            state["csb"][D : D + 1, 512:HQ],
                            state["csb"][D : D + 1, 512:HQ],
                        )

                def chunk2(c):
                    def emit():
                        # xbar transpose [DA, 128] chunks -> [128, DA].
                        # Mid-run: SP issues all (ACT is the busy engine).
                        # Last drain: ACT is idle, split issue across both
                        # HWDGE issuers to halve the ~650ns/transpose serial
                        # issue cost in the tail.
                        for i, cc in enumerate((c, c + 1)):
                            eng = nc.scalar if (last and i == 1) else nc.sync
                            eng.dma_start_transpose(
                                state["ct"][:, cc * DA : (cc + 1) * DA],
                                state["csb"][:, cc * 128 : (cc + 1) * 128],
                            )

                    return emit

                def norm2(c):
                    def emit():
                        for cc in (c, c + 1):
                            # col D of each transposed chunk = 1/denom;
                            # widen to fp32 (scalar-ptr mult needs f32)
                            nc.gpsimd.tensor_copy(
                                state["r8"][:, cc : cc + 1],
                                state["ct"][:, cc * DA + D : cc * DA + D + 1],
                            )
                            nc.gpsimd.tensor_scalar_mul(
                                state["out_sb"][:, cc * D : (cc + 1) * D],
                                state["ct"][:, cc * DA : cc * DA + D],
                                state["r8"][:, cc : cc + 1],
                            )

                    return emit

                def store():
                    if last:
                        # two SP-issued halves; the first can go while the
                        # second half's norms still run, and SP HWDGE issue
                        # is cheaper than Pool SWDGE descriptor-gen
                        nc.sync.dma_start(
                            out=out_d[b, h][:, 0 : 4 * D],
                            in_=state["out_sb"][:, 0 : 4 * D],
                        )
                        nc.sync.dma_start(
                            out=out_d[b, h][:, 4 * D : 8 * D],
                            in_=state["out_sb"][:, 4 * D : 8 * D],
                        )
                    else:
                        nc.gpsimd.dma_start(out=out_d[b, h], in_=state["out_sb"])

                return (
                    [start]
                    + [chunk2(c) for c in (0, 2, 4, 6)]
                    + [norm2(c) for c in (0, 2, 4, 6)]
                    + [store]
                )

            def body():
                pending = []  # deferred drain closures, one popped per step

                # prefetch all four batches up-front: qt/kt on the SP HWDGE
                # queue (needed first), va on the idle Pool SWDGE queue.
                # b0's kt/qt are split so the first QK can start early.
                qts, kts, vas = [], [], []
                for b in range(BPC):
                    qt_sb = io.tile([D, S], FP16, name=f"qt{b}")
                    kt_sb = io.tile([D, S], FP16, name=f"kt{b}")
                    va_sb = io.tile([128, NKT, DA], FP16, name=f"va{b}")
                    if b == 0:
                        nc.sync.dma_start(out=kt_sb[:, 0:128], in_=kt_d[b][:, 0:128])
                        nc.sync.dma_start(out=qt_sb[:, 0:512], in_=qt_d[b][:, 0:512])
                        nc.sync.dma_start(out=qt_sb[:, 512:HQ], in_=qt_d[b][:, 512:HQ])
                        nc.sync.dma_start(out=kt_sb[:, 128:S], in_=kt_d[b][:, 128:S])
                        nc.sync.dma_start(out=qt_sb[:, HQ:S], in_=qt_d[b][:, HQ:S])
                    else:
                        nc.sync.dma_start(out=kt_sb, in_=kt_d[b])
                        nc.sync.dma_start(out=qt_sb, in_=qt_d[b])
                    nc.gpsimd.dma_start(out=va_sb, in_=va_d[b])
                    qts.append(qt_sb)
                    kts.append(kt_sb)
                    vas.append(va_sb)

                av_due = []  # (due_gstep, k, pt, cx, va_sb, emitted, drain_fn)
                gstep = [0]

                def flush_av(final=False):
                    rest = []
                    due_now = []
                    for item in av_due:
                        if final or item[0] <= gstep[0]:
                            due_now.append(item)
                        else:
                            rest.append(item)
                    av_due[:] = rest
                    for _, k, pt, cx, va_sb, emitted, dr in due_now:
                        # matmul out must stay within one PSUM bank:
                        # emit per 512-col half
                        is_start = emitted[0] == 0
                        is_stop = emitted[0] == NKT - 1
                        emitted[0] += 1
                        for j in range(2):
                            nc.tensor.matmul(
                                cx[:, j * 512 : (j + 1) * 512],
                                lhsT=va_sb[:, k, :],
                                rhs=pt[:, j * 512 : (j + 1) * 512],
                                start=is_start,
                                stop=is_stop,
                                skip_group_check=True,
                            )
                        if emitted[0] == NKT:
                            # all AV writers of this cx are now registered;
                            # only now is the drain's cx->SBUF copy safe to emit
                            pending.extend(dr())

                for b in range(BPC):
                    qt_sb, kt_sb, va_sb = qts[b], kts[b], vas[b]
                    for h in range(NH):
                        cx = cxps.tile([DA, HQ], F32)
                        q0 = h * HQ
                        emitted = [0]
                        for k in range(NKT):
                            sc = scps.tile([128, HQ], F32)
                            is_dve = k in DVE_KS
                            y = dvet.tile([128, HQ], FP16, name="y16") if is_dve else None
                            for j in range(2):
                                nc.tensor.matmul(
                                    sc[:, j * 512 : (j + 1) * 512],
                                    lhsT=kt_sb[:, k * 128 : (k + 1) * 128],
                                    rhs=qt_sb[:, q0 + j * 512 : q0 + (j + 1) * 512],
                                    start=True,
                                    stop=True,
                                )
                                if is_dve:
                                    # per-half extract frees sc ASAP (2-buf
                                    # ping-pong: QK(k+2) waits on this)
                                    nc.vector.tensor_scalar_mul(
                                        y[:, j * 512 : (j + 1) * 512],
                                        sc[:, j * 512 : (j + 1) * 512],
                                        LOG2E_8,
                                    )
                            flush_av()
                            pt = ptp.tile([128, HQ], FP16)
                            if is_dve:
                                dve_exp_tail(y, pt)
                                delay = AV_DELAY_DVE
                            else:
                                nc.scalar.activation(
                                    out=pt,
                                    in_=sc,
                                    func=mybir.ActivationFunctionType.Exp,
                                    scale=0.125,
                                )
                                delay = AV_DELAY_ACT if k < 14 else AV_DELAY_LATE
                            last = b == BPC - 1 and h == NH - 1
                            av_due.append(
                                (gstep[0] + delay, k, pt, cx, va_sb, emitted,
                                 (lambda cx=cx, b=b, h=h, last=last:
                                  drain(cx, b, h, last)))
                            )
                            gstep[0] += 1
                            if pending:
                                pending.pop(0)()
                flush_av(final=True)
                for p in pending:
                    p()

            if reps == 1:
                body()
            else:
                with tc.For_i(
                    0,
                    reps,
                    1,
                    hint_engines=(
                        mybir.EngineType.PE,
                        mybir.EngineType.Activation,
                        mybir.EngineType.DVE,
                        mybir.EngineType.SP,
                        mybir.EngineType.Pool,
                    ),
                ):
                    body()

    nc.compile()
    _cache[reps] = nc
    return nc


def _prep_core_inputs(query, key, value, core):
    sl = slice(core * BPC, (core + 1) * BPC)
    qT = np.ascontiguousarray(query[sl].transpose(0, 2, 1)).astype(np.float16)
    kT = np.ascontiguousarray(key[sl].transpose(0, 2, 1)).astype(np.float16)
    v16 = value[sl].astype(np.float16)
    ones = np.ones((BPC, S, DA - D), dtype=np.float16)
    va = np.concatenate([v16, ones], axis=2)
    # [BPC, S, DA] -> [BPC, 128, NKT, DA]: row s = n*128 + p lives at [p, n]
    va_t = np.ascontiguousarray(va.reshape(BPC, NKT, 128, DA).transpose(0, 2, 1, 3))
    return {
        "qt": qT,
        "kt": kT,
        "va": va_t,
    }


def run(query, key, value, trace=False):
    nc = _build()
    query = np.asarray(query, dtype=np.float32)
    key = np.asarray(key, dtype=np.float32)
    value = np.asarray(value, dtype=np.float32)
    in_maps = [_prep_core_inputs(query, key, value, c) for c in range(NCORES)]
    res = run_bass_kernel_spmd(nc, in_maps, core_ids=list(range(NCORES)))
    outs = []
    for c in range(NCORES):
        o = np.asarray(res.results[c]["out"])  # [BPC, NH, 128, 8*D]
        o = o.reshape(BPC, NH, 128, 8, D).transpose(0, 1, 3, 2, 4).reshape(BPC, S, D)
        outs.append(o)
    return np.concatenate(outs, axis=0).astype(np.float32), res


def kernel(query, key, value):
    out, _ = run(query, key, value)
    return out


# revision 33
# speedup vs baseline: 1.0217x; 1.0217x over previous
"""Batch-parallel dot-product attention for TRN2 (8 NeuronCores).

reference: context[b] = softmax(Q[b] @ K[b].T / sqrt(64)) @ V[b]
with Q,K,V: [32, 2048, 64] fp32.

Sharding: pure data parallel - 4 batches per core, no collectives.

Per-core kernel, per (batch, 1024-query half), 16 key-tile steps:
  sc[k, q]  = (K_t @ Q^T)         one 1024-wide matmul pair per k-tile (PE)
  pt        = exp(sc/8) fp16      ACT for 13/16 steps; 3/16 steps use a
                                  7-op DVE exp2 bit-trick chain so the two
                                  engines' exp throughput adds up
  cx[d, q] += Vaug_t^T @ pt       PSUM accumulation, Vaug = [V | 1]
  (rows 64.. of cx = softmax denominator via the ones columns)

Engine layout (the whole point of this schedule):
  PE   : QK + AV matmuls only, zero-gap stream (keeps 2.4GHz p-state)
  ACT  : exp on 13/16 steps
  DVE  : exp chain on 3/16 steps + tiny reciprocal in the drain
  Pool : all drain compute (cx->SBUF fp16 copy, per-chunk normalize) + DMA
  SP   : input DMAs + xbar transposes

PSUM: sc ping-pong (2x2 banks) + cx ping-pong (2x2 banks) = 8 banks, so
AV of the next (b,h) overlaps the previous drain with no PE stall.
AV matmuls are deferred (ACT steps: 3, DVE steps: 7 global steps) so PE
never waits on the exp producing their rhs.
"""

import numpy as np

import concourse.bass as bass
import concourse.bacc as bacc
import concourse.tile as tile
from concourse import mybir
from concourse.bass_utils import run_bass_kernel_spmd

NCORES = 8
BPC = 4  # batches per core
S = 2048
D = 64
DA = 96  # V augmented to 96 cols (64 V + 32 ones) for 32-aligned xbar transpose
NKT = S // 128  # 16 key tiles of 128
NH = 2  # query halves
HQ = S // NH  # 1024 queries per half

# steps (k-tile indices) per (b,h) whose exp runs on DVE instead of ACT.
# 2/16 steps on DVE keeps ACT (14 x 996ns = 13.9us) just above the PE
# window (13.65us) while leaving DVE mostly idle so its 4.7us chains
# never congest; more DVE steps trade ACT slack for queue fragility.
DVE_KS = (4, 10)
AV_DELAY_ACT = 3  # AV of an ACT step is emitted this many global steps later
AV_DELAY_LATE = 2  # steps 14/15: release cx sooner so the drain can start
AV_DELAY_DVE = 7  # DVE chain is longer; its AVs defer further

FP16 = mybir.dt.float16
F32 = mybir.dt.float32
I16 = mybir.dt.int16

# fp16 exp2 bit-trick constants (DVE offload path): exp(s/8) = 2^y,
# y = s*log2(e)/8; t = fp16(y + 1536) rounds y to int n = t - 1536;
# s16 = t*1024 + SBIAS = fp16 bits of 2^n; g = n - y in [-0.5, 0.5];
# 2^-g ~ C0 - C1*g + C2*g^2 (minimax, rel err 2.2e-3).
LOG2E_8 = 1.4426950408889634 / 8.0
MAGIC = 1536.0
SBIAS = 15360.0 - MAGIC * 1024.0
C0, C1, C2 = 1.00053068, 0.70552215, 0.23946112

_cache = {}


def _build(reps=1):
    if reps in _cache:
        return _cache[reps]

    nc = bacc.Bacc(
        "TRN2",
        target_bir_lowering=False,
        debug=False,
        num_devices=1,
        enable_partition_id=False,
    )

    qt_d = nc.dram_tensor("qt", [BPC, D, S], FP16, kind="ExternalInput").ap()
    kt_d = nc.dram_tensor("kt", [BPC, D, S], FP16, kind="ExternalInput").ap()
    # host pre-tiles V-augmented to [BPC, 128, NKT, DA] so the DMA is contiguous
    va_d = nc.dram_tensor("va", [BPC, 128, NKT, DA], FP16, kind="ExternalInput").ap()
    # device writes [BPC, NH, 128, 8*D] contiguously; host re-tiles to [B, S, D]
    out_d = nc.dram_tensor("out", [BPC, NH, 128, 8 * D], FP16, kind="ExternalOutput").ap()

    with tile.TileContext(nc) as tc:
        with (
            tc.tile_pool(name="io", bufs=1) as io,
            tc.tile_pool(name="pt", bufs=13) as ptp,
            tc.tile_pool(name="csb", bufs=2) as csbp,
            tc.tile_pool(name="outsb", bufs=2) as outp,
            tc.tile_pool(name="dvet", bufs=3) as dvet,
            tc.tile_pool(name="scps", bufs=3, space="PSUM") as scps,
            tc.tile_pool(name="cxps", bufs=1, space="PSUM") as cxps,
        ):

            def dve_exp_tail(y, pt):
                """pt = 2^y given y (fp16 SBUF), all on DVE (tensor_scalar
                runs 4x, tensor_tensor 2x; scalar_tensor_tensor would be 1x
                so it's avoided; cross-engine hops put the drain's latency
                chain in front of pt and stall PE, so none of those).

                t rounds y to n = t - MAGIC; s16 = fp16 bits of 2^n;
                f = y - n; pt = (C2 f^2 + C1 f + C0) * 2^n.
                """
                t = dvet.tile([128, HQ], FP16, name="t16")
                nc.vector.tensor_scalar_add(t, y, MAGIC)
                n = dvet.tile([128, HQ], FP16, name="n16")
                nc.vector.tensor_scalar_sub(n, t, MAGIC)
                s16 = dvet.tile([128, HQ], I16, name="s16")
                nc.vector.tensor_scalar(
                    s16, t, 1024.0, SBIAS,
                    op0=mybir.AluOpType.mult, op1=mybir.AluOpType.add,
                )
                f = dvet.tile([128, HQ], FP16, name="f16")
                nc.vector.tensor_tensor(f, y, n, op=mybir.AluOpType.subtract)
                a = dvet.tile([128, HQ], FP16, name="a16")
                nc.vector.tensor_scalar(
                    a, f, C2, C1,
                    op0=mybir.AluOpType.mult, op1=mybir.AluOpType.add,
                )
                nc.vector.tensor_tensor(a, a, f, op=mybir.AluOpType.mult)
                nc.vector.tensor_scalar_add(a, a, C0)
                nc.vector.tensor_tensor(
                    pt, a, s16.bitcast(FP16), op=mybir.AluOpType.mult
                )

            def drain(cx, b, h, last=False):
                # split into small closures; one is emitted per k-step so
                # the work interleaves with the next half's steps. Every
                # piece is a strictly per-chunk chain (copy-half -> its
                # transposes -> per-chunk recip+norm) with no global join,
                # so no engine queue ever head-of-line blocks on a late
                # cross-engine fan-in.
                state = {}

                def start():
                    # cx -> SBUF fp16 in two halves on DVE (GPSIMD compute
                    # cannot touch PSUM on HW; ACT is the busy engine). cx
                    # is single-buffered, so the next half's first AV waits
                    # on this copy - the halves let chunk 0-3 transposes
                    # start after the first one lands.
                    state["csb"] = csbp.tile([DA, HQ], FP16, name="csb")
                    state["out_sb"] = outp.tile([128, 8 * D], FP16, name="out_sb")
                    state["ct"] = csbp.tile([128, 8 * DA], FP16, name="ctT")
                    state["r8"] = outp.tile([128, 8], F32, name="r8")
                    nc.vector.tensor_copy(state["csb"][:, 0:512], cx[:, 0:512])
                    nc.vector.tensor_copy(state["csb"][:, 512:HQ], cx[:, 512:HQ])
                    # reciprocal of the denominator row BEFORE the transpose
                    # (rows D..DA of csb are identical denom copies): one
                    # [1,1024] op; the transposes then carry 1/denom so the
                    # normalize is Pool-only with per-chunk deps. Doing this
                    # per-chunk after the transpose paced the DVE queue at
                    # transpose speed and convoyed the exp chains.
                    with nc.allow_low_precision(
                        reason="fp16 1/denom; denom O(3e3) well inside fp16, "
                        "rel err ~5e-4 vs 2e-2 gate"
                    ):
                        # in-place on row D (partition base must be 32-
                        # aligned, so row D+1 is not addressable); two
                        # halves so chunk 0-3 transposes start sooner
                        nc.vector.reciprocal(
                            state["csb"][D : D + 1, 0:512],
                            state["csb"][D : D + 1, 0:512],
                        )
                        nc.vector.reciprocal(
                            state["csb"][D : D + 1, 512:HQ],
                            state["csb"][D : D + 1, 512:HQ],
                        )

                def chunk2(c):
                    def emit():
                        # xbar transpose [DA, 128] chunks -> [128, DA].
                        # Mid-run: SP issues all (ACT is the busy engine).
                        # Last drain: ACT is idle, split issue across both
                        # HWDGE issuers to halve the ~650ns/transpose serial
                        # issue cost in the tail.
                        for i, cc in enumerate((c, c + 1)):
                            eng = nc.scalar if (last and i == 1) else nc.sync
                            eng.dma_start_transpose(
                                state["ct"][:, cc * DA : (cc + 1) * DA],
                                state["csb"][:, cc * 128 : (cc + 1) * 128],
                            )

                    return emit

                def norm2(c):
                    def emit():
                        for cc in (c, c + 1):
                            # col D of each transposed chunk = 1/denom;
                            # widen to fp32 (scalar-ptr mult needs f32)
                            nc.vector.tensor_copy(
                                state["r8"][:, cc : cc + 1],
                                state["ct"][:, cc * DA + D : cc * DA + D + 1],
                            )
                            nc.vector.tensor_scalar_mul(
                                state["out_sb"][:, cc * D : (cc + 1) * D],
                                state["ct"][:, cc * DA : cc * DA + D],
                                state["r8"][:, cc : cc + 1],
                            )

                    return emit

                def store():
                    # SP HWDGE issue; avoids Pool SWDGE descriptor-gen
                    nc.sync.dma_start(out=out_d[b, h], in_=state["out_sb"])

                return (
                    [start]
                    + [chunk2(c) for c in (0, 2, 4, 6)]
                    + [norm2(c) for c in (0, 2, 4, 6)]
                    + [store]
                )

            def body():
                pending = []  # deferred drain closures, one popped per step

                # prefetch all four batches up-front: qt/kt on the SP HWDGE
                # queue (needed first), va on the idle Pool SWDGE queue.
                # b0's kt/qt are split so the first QK can start early.
                qts, kts, vas = [], [], []
                for b in range(BPC):
                    qt_sb = io.tile([D, S], FP16, name=f"qt{b}")
                    kt_sb = io.tile([D, S], FP16, name=f"kt{b}")
                    va_sb = io.tile([128, NKT, DA], FP16, name=f"va{b}")
                    if b == 0:
                        nc.sync.dma_start(out=kt_sb[:, 0:128], in_=kt_d[b][:, 0:128])
                        nc.sync.dma_start(out=qt_sb[:, 0:512], in_=qt_d[b][:, 0:512])
                        nc.sync.dma_start(out=qt_sb[:, 512:HQ], in_=qt_d[b][:, 512:HQ])
                        nc.sync.dma_start(out=kt_sb[:, 128:S], in_=kt_d[b][:, 128:S])
                        nc.sync.dma_start(out=qt_sb[:, HQ:S], in_=qt_d[b][:, HQ:S])
                    else:
                        nc.sync.dma_start(out=kt_sb, in_=kt_d[b])
                        nc.sync.dma_start(out=qt_sb, in_=qt_d[b])
                    nc.sync.dma_start(out=va_sb, in_=va_d[b])
                    qts.append(qt_sb)
                    kts.append(kt_sb)
                    vas.append(va_sb)

                av_due = []  # (due_gstep, k, pt, cx, va_sb, emitted, drain_fn)
                gstep = [0]

                def flush_av(final=False):
                    rest = []
                    due_now = []
                    for item in av_due:
                        if final or item[0] <= gstep[0]:
                            due_now.append(item)
                        else:
                            rest.append(item)
                    av_due[:] = rest
                    for _, k, pt, cx, va_sb, emitted, dr in due_now:
                        # matmul out must stay within one PSUM bank:
                        # emit per 512-col half
                        is_start = emitted[0] == 0
                        is_stop = emitted[0] == NKT - 1
                        emitted[0] += 1
                        for j in range(2):
                            nc.tensor.matmul(
                                cx[:, j * 512 : (j + 1) * 512],
                                lhsT=va_sb[:, k, :],
                                rhs=pt[:, j * 512 : (j + 1) * 512],
                                start=is_start,
                                stop=is_stop,
                                skip_group_check=True,
                            )
                        if emitted[0] == NKT:
                            # all AV writers of this cx are now registered;
                            # only now is the drain's cx->SBUF copy safe to emit
                            pending.extend(dr())

                for b in range(BPC):
                    qt_sb, kt_sb, va_sb = qts[b], kts[b], vas[b]
                    for h in range(NH):
                        cx = cxps.tile([DA, HQ], F32)
                        q0 = h * HQ
                        emitted = [0]
                        for k in range(NKT):
                            sc = scps.tile([128, HQ], F32)
                            is_dve = k in DVE_KS
                            y = dvet.tile([128, HQ], FP16, name="y16") if is_dve else None
                            for j in range(2):
                                nc.tensor.matmul(
                                    sc[:, j * 512 : (j + 1) * 512],
                                    lhsT=kt_sb[:, k * 128 : (k + 1) * 128],
                                    rhs=qt_sb[:, q0 + j * 512 : q0 + (j + 1) * 512],
                                    start=True,
                                    stop=True,
                                )
                                if is_dve:
                                    # per-half extract frees sc ASAP (2-buf
                                    # ping-pong: QK(k+2) waits on this)
                                    nc.vector.tensor_scalar_mul(
                                        y[:, j * 512 : (j + 1) * 512],
                                        sc[:, j * 512 : (j + 1) * 512],
                                        LOG2E_8,
                                    )
                            flush_av()
                            pt = ptp.tile([128, HQ], FP16)
                            if is_dve:
                                dve_exp_tail(y, pt)
                                delay = AV_DELAY_DVE
                            else:
                                nc.scalar.activation(
                                    out=pt,
                                    in_=sc,
                                    func=mybir.ActivationFunctionType.Exp,
                                    scale=0.125,
                                )
                                delay = AV_DELAY_ACT if k < 14 else AV_DELAY_LATE
                            last = b == BPC - 1 and h == NH - 1
                            av_due.append(
                                (gstep[0] + delay, k, pt, cx, va_sb, emitted,
                                 (lambda cx=cx, b=b, h=h, last=last:
                                  drain(cx, b, h, last)))
                            )
                            gstep[0] += 1
                            if pending:
                                pending.pop(0)()
                flush_av(final=True)
                for p in pending:
                    p()

            if reps == 1:
                body()
            else:
                with tc.For_i(
                    0,
                    reps,
                    1,
                    hint_engines=(
                        mybir.EngineType.PE,
                        mybir.EngineType.Activation,
                        mybir.EngineType.DVE,
                        mybir.EngineType.SP,
                    ),
                ):
                    body()

    nc.compile()
    _cache[reps] = nc
    return nc


def _prep_core_inputs(query, key, value, core):
    sl = slice(core * BPC, (core + 1) * BPC)
    qT = np.ascontiguousarray(query[sl].transpose(0, 2, 1)).astype(np.float16)
    kT = np.ascontiguousarray(key[sl].transpose(0, 2, 1)).astype(np.float16)
    v16 = value[sl].astype(np.float16)
    ones = np.ones((BPC, S, DA - D), dtype=np.float16)
    va = np.concatenate([v16, ones], axis=2)
    # [BPC, S, DA] -> [BPC, 128, NKT, DA]: row s = n*128 + p lives at [p, n]
    va_t = np.ascontiguousarray(va.reshape(BPC, NKT, 128, DA).transpose(0, 2, 1, 3))
    return {
        "qt": qT,
        "kt": kT,
        "va": va_t,
    }


def run(query, key, value, trace=False):
    nc = _build()
    query = np.asarray(query, dtype=np.float32)
    key = np.asarray(key, dtype=np.float32)
    value = np.asarray(value, dtype=np.float32)
    in_maps = [_prep_core_inputs(query, key, value, c) for c in range(NCORES)]
    res = run_bass_kernel_spmd(nc, in_maps, core_ids=list(range(NCORES)))
    outs = []
    for c in range(NCORES):
        o = np.asarray(res.results[c]["out"])  # [BPC, NH, 128, 8*D]
        o = o.reshape(BPC, NH, 128, 8, D).transpose(0, 1, 3, 2, 4).reshape(BPC, S, D)
        outs.append(o)
    return np.concatenate(outs, axis=0).astype(np.float32), res


def kernel(query, key, value):
    out, _ = run(query, key, value)
    return out


# revision 34
# speedup vs baseline: 1.1123x; 1.0887x over previous
"""Batch-parallel dot-product attention for TRN2 (8 NeuronCores).

reference: context[b] = softmax(Q[b] @ K[b].T / sqrt(64)) @ V[b]
with Q,K,V: [32, 2048, 64] fp32.

Sharding: pure data parallel - 4 batches per core, no collectives.

Per-core kernel, per (batch, 1024-query half), 16 key-tile steps:
  sc[k, q]  = (K_t @ Q^T)         one 1024-wide matmul pair per k-tile (PE)
  pt        = exp(sc/8) fp16      ACT for 13/16 steps; 3/16 steps use a
                                  7-op DVE exp2 bit-trick chain so the two
                                  engines' exp throughput adds up
  cx[d, q] += Vaug_t^T @ pt       PSUM accumulation, Vaug = [V | 1]
  (rows 64.. of cx = softmax denominator via the ones columns)

Engine layout (the whole point of this schedule):
  PE   : QK + AV matmuls only, zero-gap stream (keeps 2.4GHz p-state)
  ACT  : exp on 13/16 steps
  DVE  : exp chain on 3/16 steps + tiny reciprocal in the drain
  Pool : all drain compute (cx->SBUF fp16 copy, per-chunk normalize) + DMA
  SP   : input DMAs + xbar transposes

PSUM: sc ping-pong (2x2 banks) + cx ping-pong (2x2 banks) = 8 banks, so
AV of the next (b,h) overlaps the previous drain with no PE stall.
AV matmuls are deferred (ACT steps: 3, DVE steps: 7 global steps) so PE
never waits on the exp producing their rhs.
"""

import numpy as np

import concourse.bass as bass
import concourse.bacc as bacc
import concourse.tile as tile
from concourse import mybir
from concourse.bass_utils import run_bass_kernel_spmd

NCORES = 8
BPC = 4  # batches per core
S = 2048
D = 64
DA = 96  # V augmented to 96 cols (64 V + 32 ones) for 32-aligned xbar transpose
NKT = S // 128  # 16 key tiles of 128
NH = 2  # query halves
HQ = S // NH  # 1024 queries per half

# steps (k-tile indices) per (b,h) whose exp runs on DVE instead of ACT.
# 2/16 steps on DVE keeps ACT (14 x 996ns = 13.9us) just above the PE
# window (13.65us) while leaving DVE mostly idle so its 4.7us chains
# never congest; more DVE steps trade ACT slack for queue fragility.
DVE_KS = (4, 10)
AV_DELAY_ACT = 3  # AV of an ACT step is emitted this many global steps later
AV_DELAY_LATE = 2  # steps 14/15: release cx sooner so the drain can start
AV_DELAY_DVE = 7  # DVE chain is longer; its AVs defer further

FP16 = mybir.dt.float16
F32 = mybir.dt.float32
I16 = mybir.dt.int16

# fp16 exp2 bit-trick constants (DVE offload path): exp(s/8) = 2^y,
# y = s*log2(e)/8; t = fp16(y + 1536) rounds y to int n = t - 1536;
# s16 = t*1024 + SBIAS = fp16 bits of 2^n; g = n - y in [-0.5, 0.5];
# 2^-g ~ C0 - C1*g + C2*g^2 (minimax, rel err 2.2e-3).
LOG2E_8 = 1.4426950408889634 / 8.0
MAGIC = 1536.0
SBIAS = 15360.0 - MAGIC * 1024.0
C0, C1, C2 = 1.00053068, 0.70552215, 0.23946112

_cache = {}


def _build(reps=1):
    if reps in _cache:
        return _cache[reps]

    nc = bacc.Bacc(
        "TRN2",
        target_bir_lowering=False,
        debug=False,
        num_devices=1,
        enable_partition_id=False,
    )

    qt_d = nc.dram_tensor("qt", [BPC, D, S], FP16, kind="ExternalInput").ap()
    kt_d = nc.dram_tensor("kt", [BPC, D, S], FP16, kind="ExternalInput").ap()
    # host pre-tiles V-augmented to [BPC, 128, NKT, DA] so the DMA is contiguous
    va_d = nc.dram_tensor("va", [BPC, 128, NKT, DA], FP16, kind="ExternalInput").ap()
    # device writes [BPC, NH, 128, 8*D] contiguously; host re-tiles to [B, S, D]
    out_d = nc.dram_tensor("out", [BPC, NH, 128, 8 * D], FP16, kind="ExternalOutput").ap()

    with tile.TileContext(nc) as tc:
        with (
            tc.tile_pool(name="io", bufs=1) as io,
            tc.tile_pool(name="pt", bufs=13) as ptp,
            tc.tile_pool(name="csb", bufs=2) as csbp,
            tc.tile_pool(name="outsb", bufs=2) as outp,
            tc.tile_pool(name="dvet", bufs=3) as dvet,
            tc.tile_pool(name="scps", bufs=3, space="PSUM") as scps,
            tc.tile_pool(name="cxps", bufs=1, space="PSUM") as cxps,
        ):

            def dve_exp_tail(y, pt):
                """pt = 2^y given y (fp16 SBUF), all on DVE (tensor_scalar
                runs 4x, tensor_tensor 2x; scalar_tensor_tensor would be 1x
                so it's avoided; cross-engine hops put the drain's latency
                chain in front of pt and stall PE, so none of those).

                t rounds y to n = t - MAGIC; s16 = fp16 bits of 2^n;
                f = y - n; pt = (C2 f^2 + C1 f + C0) * 2^n.
                """
                t = dvet.tile([128, HQ], FP16, name="t16")
                nc.vector.tensor_scalar_add(t, y, MAGIC)
                n = dvet.tile([128, HQ], FP16, name="n16")
                nc.vector.tensor_scalar_sub(n, t, MAGIC)
                s16 = dvet.tile([128, HQ], I16, name="s16")
                nc.vector.tensor_scalar(
                    s16, t, 1024.0, SBIAS,
                    op0=mybir.AluOpType.mult, op1=mybir.AluOpType.add,
                )
                f = dvet.tile([128, HQ], FP16, name="f16")
                nc.vector.tensor_tensor(f, y, n, op=mybir.AluOpType.subtract)
                a = dvet.tile([128, HQ], FP16, name="a16")
                nc.vector.tensor_scalar(
                    a, f, C2, C1,
                    op0=mybir.AluOpType.mult, op1=mybir.AluOpType.add,
                )
                nc.vector.tensor_tensor(a, a, f, op=mybir.AluOpType.mult)
                nc.vector.tensor_scalar_add(a, a, C0)
                nc.vector.tensor_tensor(
                    pt, a, s16.bitcast(FP16), op=mybir.AluOpType.mult
                )

            def drain(cx, b, h, last=False):
                # split into small closures; one is emitted per k-step so
                # the work interleaves with the next half's steps. Every
                # piece is a strictly per-chunk chain (copy-half -> its
                # transposes -> per-chunk recip+norm) with no global join,
                # so no engine queue ever head-of-line blocks on a late
                # cross-engine fan-in.
                state = {}

                def start():
                    # cx -> SBUF fp16 in two halves on DVE (GPSIMD compute
                    # cannot touch PSUM on HW; ACT is the busy engine). cx
                    # is single-buffered, so the next half's first AV waits
                    # on this copy - the halves let chunk 0-3 transposes
                    # start after the first one lands.
                    state["csb"] = csbp.tile([DA, HQ], FP16, name="csb")
                    state["out_sb"] = outp.tile([128, 8 * D], FP16, name="out_sb")
                    state["ct"] = csbp.tile([128, 8 * DA], FP16, name="ctT")
                    state["r8"] = outp.tile([128, 8], F32, name="r8")
                    nc.vector.tensor_copy(state["csb"][:, 0:512], cx[:, 0:512])
                    nc.vector.tensor_copy(state["csb"][:, 512:HQ], cx[:, 512:HQ])
                    # reciprocal of the denominator row BEFORE the transpose
                    # (rows D..DA of csb are identical denom copies): one
                    # [1,1024] op; the transposes then carry 1/denom so the
                    # normalize is Pool-only with per-chunk deps. Doing this
                    # per-chunk after the transpose paced the DVE queue at
                    # transpose speed and convoyed the exp chains.
                    with nc.allow_low_precision(
                        reason="fp16 1/denom; denom O(3e3) well inside fp16, "
                        "rel err ~5e-4 vs 2e-2 gate"
                    ):
                        # in-place on row D (partition base must be 32-
                        # aligned, so row D+1 is not addressable); two
                        # halves so chunk 0-3 transposes start sooner
                        nc.vector.reciprocal(
                            state["csb"][D : D + 1, 0:512],
                            state["csb"][D : D + 1, 0:512],
                        )
                        nc.vector.reciprocal(
                            state["csb"][D : D + 1, 512:HQ],
                            state["csb"][D : D + 1, 512:HQ],
                        )

                def chunk2(c):
                    def emit():
                        # xbar transpose [DA, 128] chunks -> [128, DA].
                        # Mid-run: SP issues all (ACT is the busy engine).
                        # Last drain: ACT is idle, split issue across both
                        # HWDGE issuers to halve the ~650ns/transpose serial
                        # issue cost in the tail.
                        for i, cc in enumerate((c, c + 1)):
                            eng = nc.scalar if (last and i == 1) else nc.sync
                            eng.dma_start_transpose(
                                state["ct"][:, cc * DA : (cc + 1) * DA],
                                state["csb"][:, cc * 128 : (cc + 1) * 128],
                            )

                    return emit

                def norm2(c):
                    def emit():
                        for cc in (c, c + 1):
                            # col D of each transposed chunk = 1/denom;
                            # widen to fp32 (scalar-ptr mult needs f32)
                            nc.vector.tensor_copy(
                                state["r8"][:, cc : cc + 1],
                                state["ct"][:, cc * DA + D : cc * DA + D + 1],
                            )
                            nc.vector.tensor_scalar_mul(
                                state["out_sb"][:, cc * D : (cc + 1) * D],
                                state["ct"][:, cc * DA : cc * DA + D],
                                state["r8"][:, cc : cc + 1],
                            )

                    return emit

                def store():
                    # SP HWDGE issue; avoids Pool SWDGE descriptor-gen
                    nc.sync.dma_start(out=out_d[b, h], in_=state["out_sb"])

                return (
                    [start]
                    + [chunk2(c) for c in (0, 2, 4, 6)]
                    + [norm2(c) for c in (0, 2, 4, 6)]
                    + [store]
                )

            def body():
                pending = []  # deferred drain closures, one popped per step

                # prefetch all four batches up-front: qt/kt on the SP HWDGE
                # queue (needed first), va on the idle Pool SWDGE queue.
                # b0's kt/qt are split so the first QK can start early.
                # qt/kt are zero-padded to 128 contraction rows: on real HW
                # a 512-col matmul with 128-row contraction is ~3x faster
                # than with 64 (measured 159ns vs 496ns). Rows 64:128 are
                # zeroed by Pool (otherwise idle) each rep; DMA fills 0:64.
                qts, kts, vas = [], [], []
                for b in range(BPC):
                    qt_sb = io.tile([128, S], FP16, name=f"qt{b}")
                    kt_sb = io.tile([128, S], FP16, name=f"kt{b}")
                    va_sb = io.tile([128, NKT, DA], FP16, name=f"va{b}")
                    nc.gpsimd.memset(kt_sb[D:128, :], 0.0)
                    nc.gpsimd.memset(qt_sb[D:128, :], 0.0)
                    if b == 0:
                        nc.sync.dma_start(
                            out=kt_sb[0:D, 0:128], in_=kt_d[b][:, 0:128]
                        )
                        nc.sync.dma_start(
                            out=qt_sb[0:D, 0:512], in_=qt_d[b][:, 0:512]
                        )
                        nc.sync.dma_start(
                            out=qt_sb[0:D, 512:HQ], in_=qt_d[b][:, 512:HQ]
                        )
                        nc.sync.dma_start(
                            out=kt_sb[0:D, 128:S], in_=kt_d[b][:, 128:S]
                        )
                        nc.sync.dma_start(
                            out=qt_sb[0:D, HQ:S], in_=qt_d[b][:, HQ:S]
                        )
                    else:
                        nc.sync.dma_start(out=kt_sb[0:D, :], in_=kt_d[b])
                        nc.sync.dma_start(out=qt_sb[0:D, :], in_=qt_d[b])
                    nc.sync.dma_start(out=va_sb, in_=va_d[b])
                    qts.append(qt_sb)
                    kts.append(kt_sb)
                    vas.append(va_sb)

                av_due = []  # (due_gstep, k, pt, cx, va_sb, emitted, drain_fn)
                gstep = [0]

                def flush_av(final=False):
                    rest = []
                    due_now = []
                    for item in av_due:
                        if final or item[0] <= gstep[0]:
                            due_now.append(item)
                        else:
                            rest.append(item)
                    av_due[:] = rest
                    for _, k, pt, cx, va_sb, emitted, dr in due_now:
                        # matmul out must stay within one PSUM bank:
                        # emit per 512-col half
                        is_start = emitted[0] == 0
                        is_stop = emitted[0] == NKT - 1
                        emitted[0] += 1
                        for j in range(2):
                            nc.tensor.matmul(
                                cx[:, j * 512 : (j + 1) * 512],
                                lhsT=va_sb[:, k, :],
                                rhs=pt[:, j * 512 : (j + 1) * 512],
                                start=is_start,
                                stop=is_stop,
                                skip_group_check=True,
                            )
                        if emitted[0] == NKT:
                            # all AV writers of this cx are now registered;
                            # only now is the drain's cx->SBUF copy safe to emit
                            pending.extend(dr())

                for b in range(BPC):
                    qt_sb, kt_sb, va_sb = qts[b], kts[b], vas[b]
                    for h in range(NH):
                        cx = cxps.tile([DA, HQ], F32)
                        q0 = h * HQ
                        emitted = [0]
                        for k in range(NKT):
                            sc = scps.tile([128, HQ], F32)
                            is_dve = k in DVE_KS
                            y = dvet.tile([128, HQ], FP16, name="y16") if is_dve else None
                            for j in range(2):
                                nc.tensor.matmul(
                                    sc[:, j * 512 : (j + 1) * 512],
                                    lhsT=kt_sb[:, k * 128 : (k + 1) * 128],
                                    rhs=qt_sb[:, q0 + j * 512 : q0 + (j + 1) * 512],
                                    start=True,
                                    stop=True,
                                )
                                if is_dve:
                                    # per-half extract frees sc ASAP (2-buf
                                    # ping-pong: QK(k+2) waits on this)
                                    nc.vector.tensor_scalar_mul(
                                        y[:, j * 512 : (j + 1) * 512],
                                        sc[:, j * 512 : (j + 1) * 512],
                                        LOG2E_8,
                                    )
                            flush_av()
                            pt = ptp.tile([128, HQ], FP16)
                            if is_dve:
                                dve_exp_tail(y, pt)
                                delay = AV_DELAY_DVE
                            else:
                                nc.scalar.activation(
                                    out=pt,
                                    in_=sc,
                                    func=mybir.ActivationFunctionType.Exp,
                                    scale=0.125,
                                )
                                delay = AV_DELAY_ACT if k < 14 else AV_DELAY_LATE
                            last = b == BPC - 1 and h == NH - 1
                            av_due.append(
                                (gstep[0] + delay, k, pt, cx, va_sb, emitted,
                                 (lambda cx=cx, b=b, h=h, last=last:
                                  drain(cx, b, h, last)))
                            )
                            gstep[0] += 1
                            if pending:
                                pending.pop(0)()
                flush_av(final=True)
                for p in pending:
                    p()

            if reps == 1:
                body()
            else:
                with tc.For_i(
                    0,
                    reps,
                    1,
                    hint_engines=(
                        mybir.EngineType.PE,
                        mybir.EngineType.Activation,
                        mybir.EngineType.DVE,
                        mybir.EngineType.SP,
                    ),
                ):
                    body()

    nc.compile()
    _cache[reps] = nc
    return nc


def _prep_core_inputs(query, key, value, core):
    sl = slice(core * BPC, (core + 1) * BPC)
    qT = np.ascontiguousarray(query[sl].transpose(0, 2, 1)).astype(np.float16)
    kT = np.ascontiguousarray(key[sl].transpose(0, 2, 1)).astype(np.float16)
    v16 = value[sl].astype(np.float16)
    ones = np.ones((BPC, S, DA - D), dtype=np.float16)
    va = np.concatenate([v16, ones], axis=2)
    # [BPC, S, DA] -> [BPC, 128, NKT, DA]: row s = n*128 + p lives at [p, n]
    va_t = np.ascontiguousarray(va.reshape(BPC, NKT, 128, DA).transpose(0, 2, 1, 3))
    return {
        "qt": qT,
        "kt": kT,
        "va": va_t,
    }


def run(query, key, value, trace=False):
    nc = _build()
    query = np.asarray(query, dtype=np.float32)
    key = np.asarray(key, dtype=np.float32)
    value = np.asarray(value, dtype=np.float32)
    in_maps = [_prep_core_inputs(query, key, value, c) for c in range(NCORES)]
    res = run_bass_kernel_spmd(nc, in_maps, core_ids=list(range(NCORES)))
    outs = []
    for c in range(NCORES):
        o = np.asarray(res.results[c]["out"])  # [BPC, NH, 128, 8*D]
        o = o.reshape(BPC, NH, 128, 8, D).transpose(0, 1, 3, 2, 4).reshape(BPC, S, D)
        outs.append(o)
    return np.concatenate(outs, axis=0).astype(np.float32), res


def kernel(query, key, value):
    out, _ = run(query, key, value)
    return out


# revision 35
# speedup vs baseline: 1.7647x; 1.5865x over previous
"""Batch-parallel dot-product attention for TRN2 (8 NeuronCores).

reference: context[b] = softmax(Q[b] @ K[b].T / sqrt(64)) @ V[b]
with Q,K,V: [32, 2048, 64] fp32.

Sharding: pure data parallel - 4 batches per core, no collectives.

Per-core kernel, per (batch, 1024-query half), 16 key-tile steps:
  sc[k, q]  = (K_t @ Q^T)         one 1024-wide matmul pair per k-tile (PE)
  pt        = exp(sc/8) fp16      ACT for 13/16 steps; 3/16 steps use a
                                  7-op DVE exp2 bit-trick chain so the two
                                  engines' exp throughput adds up
  cx[d, q] += Vaug_t^T @ pt       PSUM accumulation, Vaug = [V | 1]
  (rows 64.. of cx = softmax denominator via the ones columns)

Engine layout (the whole point of this schedule):
  PE   : QK + AV matmuls only, zero-gap stream (keeps 2.4GHz p-state)
  ACT  : exp on 13/16 steps
  DVE  : exp chain on 3/16 steps + tiny reciprocal in the drain
  Pool : all drain compute (cx->SBUF fp16 copy, per-chunk normalize) + DMA
  SP   : input DMAs + xbar transposes

PSUM: sc ping-pong (2x2 banks) + cx ping-pong (2x2 banks) = 8 banks, so
AV of the next (b,h) overlaps the previous drain with no PE stall.
AV matmuls are deferred (ACT steps: 3, DVE steps: 7 global steps) so PE
never waits on the exp producing their rhs.
"""

import numpy as np

import concourse.bass as bass
import concourse.bacc as bacc
import concourse.tile as tile
from concourse import mybir
from concourse.bass_utils import run_bass_kernel_spmd

NCORES = 8
BPC = 4  # batches per core
S = 2048
D = 64
DA = 96  # V augmented to 96 cols (64 V + 32 ones) for 32-aligned xbar transpose
NKT = S // 128  # 16 key tiles of 128
NH = 2  # query halves
HQ = S // NH  # 1024 queries per half

# steps (k-tile indices) per (b,h) whose exp runs on DVE instead of ACT.
# 2/16 steps on DVE keeps ACT (14 x 996ns = 13.9us) just above the PE
# window (13.65us) while leaving DVE mostly idle so its 4.7us chains
# never congest; more DVE steps trade ACT slack for queue fragility.
DVE_KS = ()
AV_DELAY_ACT = 3  # AV of an ACT step is emitted this many global steps later
AV_DELAY_LATE = 2  # steps 14/15: release cx sooner so the drain can start
AV_DELAY_DVE = 7  # DVE chain is longer; its AVs defer further

FP16 = mybir.dt.float16
F32 = mybir.dt.float32
I16 = mybir.dt.int16

# fp16 exp2 bit-trick constants (DVE offload path): exp(s/8) = 2^y,
# y = s*log2(e)/8; t = fp16(y + 1536) rounds y to int n = t - 1536;
# s16 = t*1024 + SBIAS = fp16 bits of 2^n; g = n - y in [-0.5, 0.5];
# 2^-g ~ C0 - C1*g + C2*g^2 (minimax, rel err 2.2e-3).
LOG2E_8 = 1.4426950408889634 / 8.0
MAGIC = 1536.0
SBIAS = 15360.0 - MAGIC * 1024.0
C0, C1, C2 = 1.00053068, 0.70552215, 0.23946112

_cache = {}


def _build(reps=1):
    if reps in _cache:
        return _cache[reps]

    nc = bacc.Bacc(
        "TRN2",
        target_bir_lowering=False,
        debug=False,
        num_devices=1,
        enable_partition_id=False,
    )

    qt_d = nc.dram_tensor("qt", [BPC, D, S], FP16, kind="ExternalInput").ap()
    kt_d = nc.dram_tensor("kt", [BPC, D, S], FP16, kind="ExternalInput").ap()
    # host pre-tiles V-augmented to [BPC, 128, NKT, DA] so the DMA is contiguous
    va_d = nc.dram_tensor("va", [BPC, 128, NKT, DA], FP16, kind="ExternalInput").ap()
    # device writes [BPC, NH, 128, 8*D] contiguously; host re-tiles to [B, S, D]
    out_d = nc.dram_tensor("out", [BPC, NH, 128, 8 * D], FP16, kind="ExternalOutput").ap()

    with tile.TileContext(nc) as tc:
        with (
            tc.tile_pool(name="io", bufs=1) as io,
            tc.tile_pool(name="pt", bufs=13) as ptp,
            tc.tile_pool(name="csb", bufs=2) as csbp,
            tc.tile_pool(name="outsb", bufs=2) as outp,
            tc.tile_pool(name="dvet", bufs=3) as dvet,
            tc.tile_pool(name="scps", bufs=3, space="PSUM") as scps,
            tc.tile_pool(name="cxps", bufs=1, space="PSUM") as cxps,
        ):

            def dve_exp_tail(y, pt):
                """pt = 2^y given y (fp16 SBUF), all on DVE (tensor_scalar
                runs 4x, tensor_tensor 2x; scalar_tensor_tensor would be 1x
                so it's avoided; cross-engine hops put the drain's latency
                chain in front of pt and stall PE, so none of those).

                t rounds y to n = t - MAGIC; s16 = fp16 bits of 2^n;
                f = y - n; pt = (C2 f^2 + C1 f + C0) * 2^n.
                """
                t = dvet.tile([128, HQ], FP16, name="t16")
                nc.vector.tensor_scalar_add(t, y, MAGIC)
                n = dvet.tile([128, HQ], FP16, name="n16")
                nc.vector.tensor_scalar_sub(n, t, MAGIC)
                s16 = dvet.tile([128, HQ], I16, name="s16")
                nc.vector.tensor_scalar(
                    s16, t, 1024.0, SBIAS,
                    op0=mybir.AluOpType.mult, op1=mybir.AluOpType.add,
                )
                f = dvet.tile([128, HQ], FP16, name="f16")
                nc.vector.tensor_tensor(f, y, n, op=mybir.AluOpType.subtract)
                a = dvet.tile([128, HQ], FP16, name="a16")
                nc.vector.tensor_scalar(
                    a, f, C2, C1,
                    op0=mybir.AluOpType.mult, op1=mybir.AluOpType.add,
                )
                nc.vector.tensor_tensor(a, a, f, op=mybir.AluOpType.mult)
                nc.vector.tensor_scalar_add(a, a, C0)
                nc.vector.tensor_tensor(
                    pt, a, s16.bitcast(FP16), op=mybir.AluOpType.mult
                )

            def drain(cx, b, h, last=False):
                # split into small closures; one is emitted per k-step so
                # the work interleaves with the next half's steps. Every
                # piece is a strictly per-chunk chain (copy-half -> its
                # transposes -> per-chunk recip+norm) with no global join,
                # so no engine queue ever head-of-line blocks on a late
                # cross-engine fan-in.
                state = {}

                def start():
                    # cx -> SBUF fp16 in two halves on DVE (GPSIMD compute
                    # cannot touch PSUM on HW; ACT is the busy engine). cx
                    # is single-buffered, so the next half's first AV waits
                    # on this copy - the halves let chunk 0-3 transposes
                    # start after the first one lands.
                    state["csb"] = csbp.tile([DA, HQ], FP16, name="csb")
                    state["out_sb"] = outp.tile([128, 8 * D], FP16, name="out_sb")
                    state["ct"] = csbp.tile([128, 8 * DA], FP16, name="ctT")
                    state["r8"] = outp.tile([128, 8], F32, name="r8")
                    nc.vector.tensor_copy(state["csb"][:, 0:512], cx[:, 0:512])
                    nc.vector.tensor_copy(state["csb"][:, 512:HQ], cx[:, 512:HQ])
                    # reciprocal of the denominator row BEFORE the transpose
                    # (rows D..DA of csb are identical denom copies): one
                    # [1,1024] op; the transposes then carry 1/denom so the
                    # normalize is Pool-only with per-chunk deps. Doing this
                    # per-chunk after the transpose paced the DVE queue at
                    # transpose speed and convoyed the exp chains.
                    with nc.allow_low_precision(
                        reason="fp16 1/denom; denom O(3e3) well inside fp16, "
                        "rel err ~5e-4 vs 2e-2 gate"
                    ):
                        # in-place on row D (partition base must be 32-
                        # aligned, so row D+1 is not addressable); two
                        # halves so chunk 0-3 transposes start sooner
                        nc.vector.reciprocal(
                            state["csb"][D : D + 1, 0:512],
                            state["csb"][D : D + 1, 0:512],
                        )
                        nc.vector.reciprocal(
                            state["csb"][D : D + 1, 512:HQ],
                            state["csb"][D : D + 1, 512:HQ],
                        )

                def chunk2(c):
                    def emit():
                        # xbar transpose [DA, 128] chunks -> [128, DA].
                        # Mid-run: SP issues all (ACT is the busy engine).
                        # Last drain: ACT is idle, split issue across both
                        # HWDGE issuers to halve the ~650ns/transpose serial
                        # issue cost in the tail.
                        for i, cc in enumerate((c, c + 1)):
                            eng = nc.scalar if (last and i == 1) else nc.sync
                            eng.dma_start_transpose(
                                state["ct"][:, cc * DA : (cc + 1) * DA],
                                state["csb"][:, cc * 128 : (cc + 1) * 128],
                            )

                    return emit

                def norm2(c):
                    def emit():
                        for cc in (c, c + 1):
                            # col D of each transposed chunk = 1/denom;
                            # widen to fp32 (scalar-ptr mult needs f32)
                            nc.vector.tensor_copy(
                                state["r8"][:, cc : cc + 1],
                                state["ct"][:, cc * DA + D : cc * DA + D + 1],
                            )
                            nc.vector.tensor_scalar_mul(
                                state["out_sb"][:, cc * D : (cc + 1) * D],
                                state["ct"][:, cc * DA : cc * DA + D],
                                state["r8"][:, cc : cc + 1],
                            )

                    return emit

                def store():
                    # SP HWDGE issue; avoids Pool SWDGE descriptor-gen
                    nc.sync.dma_start(out=out_d[b, h], in_=state["out_sb"])

                return (
                    [start]
                    + [chunk2(c) for c in (0, 2, 4, 6)]
                    + [norm2(c) for c in (0, 2, 4, 6)]
                    + [store]
                )

            def body():
                pending = []  # deferred drain closures, one popped per step

                # prefetch all four batches up-front: qt/kt on the SP HWDGE
                # queue (needed first), va on the idle Pool SWDGE queue.
                # b0's kt/qt are split so the first QK can start early.
                # qt/kt are zero-padded to 128 contraction rows: on real HW
                # a 512-col matmul with 128-row contraction is ~3x faster
                # than with 64 (measured 159ns vs 496ns). Rows 64:128 are
                # zeroed by Pool (otherwise idle) each rep; DMA fills 0:64.
                qts, kts, vas = [], [], []
                for b in range(BPC):
                    qt_sb = io.tile([128, S], FP16, name=f"qt{b}")
                    kt_sb = io.tile([128, S], FP16, name=f"kt{b}")
                    va_sb = io.tile([128, NKT, DA], FP16, name=f"va{b}")
                    nc.gpsimd.memset(kt_sb[D:128, :], 0.0)
                    nc.gpsimd.memset(qt_sb[D:128, :], 0.0)
                    if b == 0:
                        nc.sync.dma_start(
                            out=kt_sb[0:D, 0:128], in_=kt_d[b][:, 0:128]
                        )
                        nc.sync.dma_start(
                            out=qt_sb[0:D, 0:512], in_=qt_d[b][:, 0:512]
                        )
                        nc.sync.dma_start(
                            out=qt_sb[0:D, 512:HQ], in_=qt_d[b][:, 512:HQ]
                        )
                        nc.sync.dma_start(
                            out=kt_sb[0:D, 128:S], in_=kt_d[b][:, 128:S]
                        )
                        nc.sync.dma_start(
                            out=qt_sb[0:D, HQ:S], in_=qt_d[b][:, HQ:S]
                        )
                    else:
                        nc.sync.dma_start(out=kt_sb[0:D, :], in_=kt_d[b])
                        nc.sync.dma_start(out=qt_sb[0:D, :], in_=qt_d[b])
                    nc.sync.dma_start(out=va_sb, in_=va_d[b])
                    qts.append(qt_sb)
                    kts.append(kt_sb)
                    vas.append(va_sb)

                av_due = []  # (due_gstep, k, pt, cx, va_sb, emitted, drain_fn)
                gstep = [0]

                def flush_av(final=False):
                    rest = []
                    due_now = []
                    for item in av_due:
                        if final or item[0] <= gstep[0]:
                            due_now.append(item)
                        else:
                            rest.append(item)
                    av_due[:] = rest
                    for _, k, pt, cx, va_sb, emitted, dr in due_now:
                        # matmul out must stay within one PSUM bank:
                        # emit per 512-col half
                        is_start = emitted[0] == 0
                        is_stop = emitted[0] == NKT - 1
                        emitted[0] += 1
                        for j in range(2):
                            nc.tensor.matmul(
                                cx[:, j * 512 : (j + 1) * 512],
                                lhsT=va_sb[:, k, :],
                                rhs=pt[:, j * 512 : (j + 1) * 512],
                                start=is_start,
                                stop=is_stop,
                                skip_group_check=True,
                            )
                        if emitted[0] == NKT:
                            # all AV writers of this cx are now registered;
                            # only now is the drain's cx->SBUF copy safe to emit
                            pending.extend(dr())

                for b in range(BPC):
                    qt_sb, kt_sb, va_sb = qts[b], kts[b], vas[b]
                    for h in range(NH):
                        cx = cxps.tile([DA, HQ], F32)
                        q0 = h * HQ
                        emitted = [0]
                        for k in range(NKT):
                            sc = scps.tile([128, HQ], F32)
                            is_dve = k in DVE_KS
                            y = dvet.tile([128, HQ], FP16, name="y16") if is_dve else None
                            for j in range(2):
                                nc.tensor.matmul(
                                    sc[:, j * 512 : (j + 1) * 512],
                                    lhsT=kt_sb[:, k * 128 : (k + 1) * 128],
                                    rhs=qt_sb[:, q0 + j * 512 : q0 + (j + 1) * 512],
                                    start=True,
                                    stop=True,
                                )
                                if is_dve:
                                    # per-half extract frees sc ASAP (2-buf
                                    # ping-pong: QK(k+2) waits on this)
                                    nc.vector.tensor_scalar_mul(
                                        y[:, j * 512 : (j + 1) * 512],
                                        sc[:, j * 512 : (j + 1) * 512],
                                        LOG2E_8,
                                    )
                            flush_av()
                            pt = ptp.tile([128, HQ], FP16)
                            if is_dve:
                                dve_exp_tail(y, pt)
                                delay = AV_DELAY_DVE
                            else:
                                nc.scalar.activation(
                                    out=pt,
                                    in_=sc,
                                    func=mybir.ActivationFunctionType.Exp,
                                    scale=0.125,
                                )
                                delay = AV_DELAY_ACT if k < 14 else AV_DELAY_LATE
                            last = b == BPC - 1 and h == NH - 1
                            av_due.append(
                                (gstep[0] + delay, k, pt, cx, va_sb, emitted,
                                 (lambda cx=cx, b=b, h=h, last=last:
                                  drain(cx, b, h, last)))
                            )
                            gstep[0] += 1
                            if pending:
                                pending.pop(0)()
                flush_av(final=True)
                for p in pending:
                    p()

            if reps == 1:
                body()
            else:
                with tc.For_i(
                    0,
                    reps,
                    1,
                    hint_engines=(
                        mybir.EngineType.PE,
                        mybir.EngineType.Activation,
                        mybir.EngineType.DVE,
                        mybir.EngineType.SP,
                    ),
                ):
                    body()

    nc.compile()
    _cache[reps] = nc
    return nc


def _prep_core_inputs(query, key, value, core):
    sl = slice(core * BPC, (core + 1) * BPC)
    qT = np.ascontiguousarray(query[sl].transpose(0, 2, 1)).astype(np.float16)
    kT = np.ascontiguousarray(key[sl].transpose(0, 2, 1)).astype(np.float16)
    v16 = value[sl].astype(np.float16)
    ones = np.ones((BPC, S, DA - D), dtype=np.float16)
    va = np.concatenate([v16, ones], axis=2)
    # [BPC, S, DA] -> [BPC, 128, NKT, DA]: row s = n*128 + p lives at [p, n]
    va_t = np.ascontiguousarray(va.reshape(BPC, NKT, 128, DA).transpose(0, 2, 1, 3))
    return {
        "qt": qT,
        "kt": kT,
        "va": va_t,
    }


def run(query, key, value, trace=False):
    nc = _build()
    query = np.asarray(query, dtype=np.float32)
    key = np.asarray(key, dtype=np.float32)
    value = np.asarray(value, dtype=np.float32)
    in_maps = [_prep_core_inputs(query, key, value, c) for c in range(NCORES)]
    res = run_bass_kernel_spmd(nc, in_maps, core_ids=list(range(NCORES)))
    outs = []
    for c in range(NCORES):
        o = np.asarray(res.results[c]["out"])  # [BPC, NH, 128, 8*D]
        o = o.reshape(BPC, NH, 128, 8, D).transpose(0, 1, 3, 2, 4).reshape(BPC, S, D)
        outs.append(o)
    return np.concatenate(outs, axis=0).astype(np.float32), res


def kernel(query, key, value):
    out, _ = run(query, key, value)
    return out
